# revision 8
# baseline (speedup 1.0000x reference)
"""EngagementBiasedMHA on 8 Trainium2 NeuronCores.

Sharding: 4 batches x 2 head-groups (8 heads each).  Each core computes, for
its (batch, head-group):
  - Q^T/K^T projections in [feat, token] layout (feature dim on partitions)
  - V projection in [token, feat] layout, stored per key-tile as
    [ones(64) | V_h] so the PV matmul also produces the softmax denominator
    on partitions 0:64 (o on 64:128)
  - attention in transposed layout: S^T = K @ Q^T with keys on partitions, so
    the per-key engagement bias/mask folds into the Exp activation as a
    per-partition bias, and exp(S^T) is already the correct (lhs-contraction)
    layout for the PV matmul
  - O^T = Vhat^T @ P^T accumulated over key tiles (rows 0:64 = replicated
    softmax denominator, rows 64:128 = head output)
  - row-parallel partial output projection y_partial = O_hg @ out_w.T[hg]
Matmul operands are bf16 (4x PE throughput vs fp32); accumulation stays fp32.
Host side: transpose/slice inputs per core, then sum the two partial outputs
per batch (row-parallel unshard).
"""

import sys

if "/opt/trn_rl_repo" not in sys.path:
    sys.path.insert(0, "/opt/trn_rl_repo")

import numpy as np
from concourse import bacc, tile
import concourse.mybir as mybir
from concourse.bass_utils import run_bass_kernel_spmd

F32 = mybir.dt.float32
BF16 = mybir.dt.bfloat16
NP_BF16 = mybir.dt.np(BF16)
AF = mybir.ActivationFunctionType

B, T, D, H = 4, 2048, 1024, 16
HD = 64
HG = 8           # heads per core
NKT = T // 128   # 16 key/token tiles
NQC = T // 512   # 4 query chunks
NDT = D // 128   # 8 d_in tiles
VROW = HG * 128  # 1024 Vhat columns per key tile: per head [V(64) | ones(64)]

_cache = {}

# Results of the most recent run (for the test harness to read exec times).
last_results = None


def _build_program():
    nc = bacc.Bacc("TRN2", target_bir_lowering=False, debug=False, num_devices=8)
    xt_d = nc.declare_dram_parameter("xt", [D, T], BF16, isOutput=False)
    # wqk: row block m*128+p holds, at col d*128+f, weight qkv_w.T[d*128+p, feat(m)+f]
    wqk_d = nc.declare_dram_parameter("wqk", [1024, 1024], BF16, isOutput=False)
    wv_d = nc.declare_dram_parameter("wv", [D, 512], BF16, isOutput=False)
    bqk_d = nc.declare_dram_parameter("bqk", [128, 8], F32, isOutput=False)
    bv_d = nc.declare_dram_parameter("bv", [128, 512], F32, isOutput=False)
    eng_d = nc.declare_dram_parameter("eng", [128, NKT], F32, isOutput=False)
    maskf_d = nc.declare_dram_parameter("maskf", [128, NKT], F32, isOutput=False)
    wo_d = nc.declare_dram_parameter("wo", [512, 1024], BF16, isOutput=False)
    bo_d = nc.declare_dram_parameter("bo", [128, 1024], F32, isOutput=False)
    y_d = nc.declare_dram_parameter("y", [T, D], F32, isOutput=True)

    with tile.TileContext(nc) as tc:
        with (
            tc.tile_pool(name="persist", bufs=1) as persist,
            tc.tile_pool(name="xtpool", bufs=2) as xtpool,
            tc.tile_pool(name="wqkpool", bufs=2) as wqkpool,
            tc.tile_pool(name="wpool", bufs=1) as wpool,
            tc.tile_pool(name="small", bufs=1) as small,
            tc.tile_pool(name="ptpool", bufs=4) as ptpool,
            tc.tile_pool(name="otpool", bufs=6) as otpool,
            tc.tile_pool(name="evacpool", bufs=3) as evacpool,
            tc.tile_pool(name="recpool", bufs=3) as recpool,
            tc.tile_pool(name="psmix", bufs=4, space="PSUM") as psmix,
            tc.tile_pool(name="psST", bufs=2, space="PSUM") as psST,
        ):
            # ---- small inputs ----
            BQK = small.tile([128, 8], F32, name="BQK")
            nc.sync.dma_start(BQK[:], bqk_d[:])
            BV = small.tile([128, 512], F32, name="BV")
            nc.sync.dma_start(BV[:], bv_d[:])
            ENG = small.tile([128, NKT], F32, name="ENG")
            nc.sync.dma_start(ENG[:], eng_d[:])
            MSK = small.tile([128, NKT], F32, name="MSK")
            nc.sync.dma_start(MSK[:], maskf_d[:])
            BO = small.tile([128, 1024], F32, name="BO")
            nc.sync.dma_start(BO[:], bo_d[:])

            # ---- per-key bias: BK = ln(max(eng, 1e-6)) - 1e9 * mask ----
            BK = small.tile([128, NKT], F32, name="BK")
            nc.vector.tensor_scalar_max(BK[:], ENG[:], 1e-6)
            nc.scalar.activation(BK[:], BK[:], AF.Ln)
            MK9 = small.tile([128, NKT], F32, name="MK9")
            nc.vector.tensor_scalar_mul(MK9[:], MSK[:], -1e9)
            nc.vector.tensor_add(BK[:], BK[:], MK9[:])

            # ---- phase 1: projections, chunked over 512-token chunks ----
            QTKT = persist.tile([128, 8 * T], BF16, name="QTKT")
            VHAT = persist.tile([128, NKT * VROW], BF16, name="VHAT")
            nc.gpsimd.memset(VHAT[:], 1.0)
            WV = wpool.tile([128, NDT * 512], BF16, name="WV", tag="wv_wo")
            for d in range(NDT):
                nc.sync.dma_start(WV[:, d * 512:(d + 1) * 512], wv_d[d * 128:(d + 1) * 128, :])

            for c in range(NQC):
                XTc = xtpool.tile([128, NDT * 512], BF16, name="XTc", tag="xtc")
                for d in range(NDT):
                    nc.sync.dma_start(XTc[:, d * 512:(d + 1) * 512],
                                      xt_d[d * 128:(d + 1) * 128, c * 512:(c + 1) * 512])
                # Q^T / K^T features (8 tiles of 128 feats each)
                for m in range(8):
                    WQKm = wqkpool.tile([128, 1024], BF16, name="WQKm", tag="wqk")
                    nc.sync.dma_start(WQKm[:], wqk_d[m * 128:(m + 1) * 128, :])
                    ps = psmix.tile([128, 512], F32, name="ps_qk", tag="mix")
                    for d in range(NDT):
                        nc.tensor.matmul(
                            ps[:],
                            lhsT=WQKm[:, d * 128:(d + 1) * 128],
                            rhs=XTc[:, d * 512:(d + 1) * 512],
                            start=(d == 0), stop=(d == NDT - 1),
                        )
                    nc.scalar.activation(
                        QTKT[:, m * T + c * 512: m * T + c * 512 + 512],
                        ps[:], AF.Identity, bias=BQK[:, m:m + 1])
                # V for this chunk's 4 token tiles
                for t4 in range(4):
                    t = c * 4 + t4
                    ps = psmix.tile([128, 512], F32, name="ps_v", tag="mix")
                    for d in range(NDT):
                        nc.tensor.matmul(
                            ps[:],
                            lhsT=XTc[:, d * 512 + t4 * 128: d * 512 + (t4 + 1) * 128],
                            rhs=WV[:, d * 512:(d + 1) * 512],
                            start=(d == 0), stop=(d == NDT - 1),
                        )
                    vslice = VHAT[:, t * VROW:(t + 1) * VROW].rearrange(
                        "p (h c) -> p h c", c=128)[:, :, 64:128]
                    nc.vector.tensor_add(
                        vslice,
                        ps[:].rearrange("p (h c) -> p h c", c=64),
                        BV[:].rearrange("p (h c) -> p h c", c=64))

            WO = wpool.tile([128, 4 * 1024], BF16, name="WO", tag="wv_wo")
            for f in range(4):
                nc.sync.dma_start(WO[:, f * 1024:(f + 1) * 1024], wo_d[f * 128:(f + 1) * 128, :])

            # ---- phase 2+3: attention (transposed layout) + output projection ----
            for qc in range(NQC):
                otc = []
                for hp in range(4):
                    qt = hp
                    ktf = 4 + hp
                    op0 = psmix.tile([128, 512], F32, name="op0", tag="mix")
                    op1 = psmix.tile([128, 512], F32, name="op1", tag="mix")
                    ops = (op0, op1)
                    for kt in range(NKT):
                        st = psST.tile([128, 1024], F32, name="st", tag="st")
                        for sub in range(2):
                            lo = sub * 64
                            nc.tensor.matmul(
                                st[:, sub * 512:(sub + 1) * 512],
                                lhsT=QTKT[lo:lo + 64, ktf * T + kt * 128: ktf * T + (kt + 1) * 128],
                                rhs=QTKT[lo:lo + 64, qt * T + qc * 512: qt * T + qc * 512 + 512],
                                start=True, stop=True)
                        pt = ptpool.tile([128, 1024], BF16, name="pt", tag="pt")
                        nc.scalar.activation(
                            pt[:], st[:], AF.Exp,
                            bias=BK[:, kt:kt + 1], scale=0.125)
                        for sub in range(2):
                            h = 2 * hp + sub
                            nc.tensor.matmul(
                                ops[sub][:],
                                lhsT=VHAT[:, kt * VROW + h * 128: kt * VROW + (h + 1) * 128],
                                rhs=pt[:, sub * 512:(sub + 1) * 512],
                                start=(kt == 0), stop=(kt == NKT - 1))
                    OTc = otpool.tile([128, 512], BF16, name="OTc", tag="otc")
                    for sub in range(2):
                        rec = recpool.tile([64, 512], F32, name="rec", tag="rec")
                        nc.vector.reciprocal_approx_fast(rec[:], ops[sub][0:64, :])
                        nc.vector.tensor_mul(
                            OTc[sub * 64:sub * 64 + 64, :],
                            ops[sub][64:128, :], rec[:])
                    otc.append(OTc)
                # output projection for this 512-token chunk
                for t4 in range(4):
                    tt = qc * 4 + t4
                    for c2 in range(2):
                        ps = psmix.tile([128, 512], F32, name="ps_y", tag="mix")
                        for f in range(4):
                            nc.tensor.matmul(
                                ps[:],
                                lhsT=otc[f][:, t4 * 128:(t4 + 1) * 128],
                                rhs=WO[:, f * 1024 + c2 * 512: f * 1024 + c2 * 512 + 512],
                                start=(f == 0), stop=(f == 3))
                        yv = evacpool.tile([128, 512], F32, name="yv", tag="yv")
                        nc.vector.tensor_add(yv[:], ps[:], BO[:, c2 * 512:(c2 + 1) * 512])
                        nc.sync.dma_start(
                            y_d[tt * 128:(tt + 1) * 128, c2 * 512:(c2 + 1) * 512], yv[:])

    nc.compile()
    return nc


def get_program():
    if "nc" not in _cache:
        _cache["nc"] = _build_program()
    return _cache["nc"]


def shard_inputs(x, engagement, mask, qkv_w, qkv_b, out_w, out_b):
    """Build the per-core input maps (host-side layout prep only)."""
    x = np.asarray(x, dtype=np.float32)
    engagement = np.asarray(engagement, dtype=np.float32)
    maskf = np.asarray(mask).astype(np.float32)
    qkv_w = np.asarray(qkv_w, dtype=np.float32)
    qkv_b = np.asarray(qkv_b, dtype=np.float32)
    out_w = np.asarray(out_w, dtype=np.float32)
    out_b = np.asarray(out_b, dtype=np.float32)

    qkvT = qkv_w.T  # [D, 3D]
    outT = out_w.T  # [D, D]
    in_maps = []
    for cix in range(8):
        b, hg = cix // 2, cix % 2
        qcols = qkvT[:, hg * 512:(hg + 1) * 512]
        kcols = qkvT[:, 1024 + hg * 512: 1024 + (hg + 1) * 512]
        sel = np.concatenate([qcols, kcols], axis=1)  # [1024 din, 1024 feats]
        # [d, p, m, f] -> [m, p, d, f] -> [(m p), (d f)]
        wqk = sel.reshape(NDT, 128, 8, 128).transpose(2, 1, 0, 3).reshape(1024, 1024)
        bq = qkv_b[hg * 512:(hg + 1) * 512].reshape(4, 128).T
        bk = qkv_b[1024 + hg * 512: 1024 + (hg + 1) * 512].reshape(4, 128).T
        bo = np.broadcast_to(out_b, (128, 1024)) if hg == 0 else np.zeros((128, 1024), np.float32)
        in_maps.append({
            "xt": np.ascontiguousarray(x[b].T).astype(NP_BF16),
            "wqk": np.ascontiguousarray(wqk).astype(NP_BF16),
            "wv": np.ascontiguousarray(
                qkvT[:, 2048 + hg * 512: 2048 + (hg + 1) * 512]).astype(NP_BF16),
            "bqk": np.ascontiguousarray(np.concatenate([bq, bk], axis=1)),
            "bv": np.ascontiguousarray(
                np.broadcast_to(qkv_b[2048 + hg * 512: 2048 + (hg + 1) * 512], (128, 512))),
            "eng": np.ascontiguousarray(engagement[b].reshape(NKT, 128).T),
            "maskf": np.ascontiguousarray(maskf[b].reshape(NKT, 128).T),
            "wo": np.ascontiguousarray(outT[hg * 512:(hg + 1) * 512, :]).astype(NP_BF16),
            "bo": np.ascontiguousarray(bo),
        })
    return in_maps


def kernel(x, engagement, mask, qkv_w, qkv_b, out_w, out_b):
    global last_results
    nc = get_program()
    in_maps = shard_inputs(x, engagement, mask, qkv_w, qkv_b, out_w, out_b)
    res = run_bass_kernel_spmd(nc, in_maps, list(range(8)))
    last_results = res
    out = np.empty((B, T, D), dtype=np.float32)
    for b in range(B):
        out[b] = res.results[2 * b]["y"] + res.results[2 * b + 1]["y"]
    return out


# revision 9
# speedup vs baseline: 1.0091x; 1.0091x over previous
"""EngagementBiasedMHA on 8 Trainium2 NeuronCores.

Sharding: 4 batches x 2 head-groups (8 heads each).  Each core computes, for
its (batch, head-group):
  - K^T projection in [feat, token] layout and V projection in [token, feat]
    layout (phase 1); V is stored per key-tile as [ones(64) | V_h] so the PV
    matmul also produces the softmax denominator on partitions 0:64
  - per 512-query chunk: Q^T projection (overlapped with attention of the
    previous chunk), then attention in transposed layout: S^T = K @ Q^T with
    keys on partitions, so the per-key engagement bias/mask folds into the
    Exp activation as a per-partition bias, and exp(S^T) is already the
    correct (lhs-contraction) layout for the PV matmul
  - O^T = Vhat^T @ P^T accumulated over key tiles (rows 0:64 = replicated
    softmax denominator, rows 64:128 = head output)
  - row-parallel partial output projection y_partial = O_hg @ out_w.T[hg]
Matmul operands are bf16 (4x PE throughput vs fp32); accumulation stays fp32.
Host side: transpose/slice inputs per core, then sum the two partial outputs
per batch (row-parallel unshard).
"""

import sys

if "/opt/trn_rl_repo" not in sys.path:
    sys.path.insert(0, "/opt/trn_rl_repo")

import numpy as np
from concourse import bacc, tile
import concourse.mybir as mybir
from concourse.bass_utils import run_bass_kernel_spmd

F32 = mybir.dt.float32
BF16 = mybir.dt.bfloat16
NP_BF16 = mybir.dt.np(BF16)
AF = mybir.ActivationFunctionType

B, T, D, H = 4, 2048, 1024, 16
HD = 64
HG = 8           # heads per core
NKT = T // 128   # 16 key/token tiles
NQC = T // 512   # 4 query chunks
NDT = D // 128   # 8 d_in tiles
VROW = HG * 128  # 1024 Vhat columns per key tile: per head [ones(64) | V(64)]

_cache = {}

# Results of the most recent run (for the test harness to read exec times).
last_results = None


def _build_program():
    nc = bacc.Bacc("TRN2", target_bir_lowering=False, debug=False, num_devices=8)
    xt_d = nc.declare_dram_parameter("xt", [D, T], BF16, isOutput=False)
    # wqk: row block m*128+p holds, at col d*128+f, weight qkv_w.T[d*128+p, feat(m)+f]
    wqk_d = nc.declare_dram_parameter("wqk", [1024, 1024], BF16, isOutput=False)
    wv_d = nc.declare_dram_parameter("wv", [D, 512], BF16, isOutput=False)
    bqk_d = nc.declare_dram_parameter("bqk", [128, 8], F32, isOutput=False)
    bv_d = nc.declare_dram_parameter("bv", [128, 512], F32, isOutput=False)
    eng_d = nc.declare_dram_parameter("eng", [128, NKT], F32, isOutput=False)
    maskf_d = nc.declare_dram_parameter("maskf", [128, NKT], F32, isOutput=False)
    wo_d = nc.declare_dram_parameter("wo", [512, 1024], BF16, isOutput=False)
    bo_d = nc.declare_dram_parameter("bo", [128, 1024], F32, isOutput=False)
    y_d = nc.declare_dram_parameter("y", [T, D], F32, isOutput=True)

    with tile.TileContext(nc) as tc:
        with (
            tc.tile_pool(name="persist", bufs=1) as persist,
            tc.tile_pool(name="wpool", bufs=1) as wpool,
            tc.tile_pool(name="small", bufs=1) as small,
            tc.tile_pool(name="ptpool", bufs=4) as ptpool,
            tc.tile_pool(name="otpool", bufs=6) as otpool,
            tc.tile_pool(name="evacpool", bufs=3) as evacpool,
            tc.tile_pool(name="recpool", bufs=3) as recpool,
            tc.tile_pool(name="psmix", bufs=4, space="PSUM") as psmix,
            tc.tile_pool(name="psST", bufs=2, space="PSUM") as psST,
        ):
            # ---- small inputs ----
            BQK = small.tile([128, 8], F32, name="BQK")
            nc.sync.dma_start(BQK[:], bqk_d[:])
            BV = small.tile([128, 512], F32, name="BV")
            nc.sync.dma_start(BV[:], bv_d[:])
            ENG = small.tile([128, NKT], F32, name="ENG")
            nc.sync.dma_start(ENG[:], eng_d[:])
            MSK = small.tile([128, NKT], F32, name="MSK")
            nc.sync.dma_start(MSK[:], maskf_d[:])
            BO = small.tile([128, 1024], F32, name="BO")
            nc.sync.dma_start(BO[:], bo_d[:])

            # ---- per-key bias: BK = ln(max(eng, 1e-6)) - 1e9 * mask ----
            BK = small.tile([128, NKT], F32, name="BK")
            nc.vector.tensor_scalar_max(BK[:], ENG[:], 1e-6)
            nc.scalar.activation(BK[:], BK[:], AF.Ln)
            MK9 = small.tile([128, NKT], F32, name="MK9")
            nc.vector.tensor_scalar_mul(MK9[:], MSK[:], -1e9)
            nc.vector.tensor_add(BK[:], BK[:], MK9[:])

            # ---- resident activations / weights (bf16) ----
            XT = persist.tile([128, NDT * T], BF16, name="XT")
            for d in range(NDT):
                nc.sync.dma_start(XT[:, d * T:(d + 1) * T], xt_d[d * 128:(d + 1) * 128, :])
            WQK = persist.tile([128, 8 * 1024], BF16, name="WQK")
            for m in range(8):
                nc.sync.dma_start(WQK[:, m * 1024:(m + 1) * 1024],
                                  wqk_d[m * 128:(m + 1) * 128, :])
            WV = wpool.tile([128, NDT * 512], BF16, name="WV", tag="wv_wo")
            for d in range(NDT):
                nc.sync.dma_start(WV[:, d * 512:(d + 1) * 512], wv_d[d * 128:(d + 1) * 128, :])

            QTKT = persist.tile([128, 8 * T], BF16, name="QTKT")
            VHAT = persist.tile([128, NKT * VROW], BF16, name="VHAT")
            nc.gpsimd.memset(VHAT[:], 1.0)

            # ---- phase 1: K^T and V projections (chunked over tokens) ----
            for c in range(NQC):
                for m in range(4, 8):  # K feature tiles
                    ps = psmix.tile([128, 512], F32, name="ps_k", tag="mix")
                    for d in range(NDT):
                        nc.tensor.matmul(
                            ps[:],
                            lhsT=WQK[:, m * 1024 + d * 128: m * 1024 + (d + 1) * 128],
                            rhs=XT[:, d * T + c * 512: d * T + c * 512 + 512],
                            start=(d == 0), stop=(d == NDT - 1),
                        )
                    nc.scalar.activation(
                        QTKT[:, m * T + c * 512: m * T + c * 512 + 512],
                        ps[:], AF.Identity, bias=BQK[:, m:m + 1])
                for t4 in range(4):
                    t = c * 4 + t4
                    ps = psmix.tile([128, 512], F32, name="ps_v", tag="mix")
                    for d in range(NDT):
                        nc.tensor.matmul(
                            ps[:],
                            lhsT=XT[:, d * T + t * 128: d * T + (t + 1) * 128],
                            rhs=WV[:, d * 512:(d + 1) * 512],
                            start=(d == 0), stop=(d == NDT - 1),
                        )
                    vslice = VHAT[:, t * VROW:(t + 1) * VROW].rearrange(
                        "p (h c) -> p h c", c=128)[:, :, 64:128]
                    nc.vector.tensor_add(
                        vslice,
                        ps[:].rearrange("p (h c) -> p h c", c=64),
                        BV[:].rearrange("p (h c) -> p h c", c=64))

            WO = wpool.tile([128, 4 * 1024], BF16, name="WO", tag="wv_wo")
            for f in range(4):
                nc.sync.dma_start(WO[:, f * 1024:(f + 1) * 1024], wo_d[f * 128:(f + 1) * 128, :])

            # ---- phase 2: per query chunk: Q^T projection + attention + out-proj ----
            for qc in range(NQC):
                # Q^T projection for this chunk (overlaps previous chunk's attention)
                for m in range(4):
                    ps = psmix.tile([128, 512], F32, name="ps_q", tag="mix")
                    for d in range(NDT):
                        nc.tensor.matmul(
                            ps[:],
                            lhsT=WQK[:, m * 1024 + d * 128: m * 1024 + (d + 1) * 128],
                            rhs=XT[:, d * T + qc * 512: d * T + qc * 512 + 512],
                            start=(d == 0), stop=(d == NDT - 1),
                        )
                    nc.vector.tensor_scalar_add(
                        QTKT[:, m * T + qc * 512: m * T + qc * 512 + 512],
                        ps[:], BQK[:, m:m + 1])

                otc = []
                for hp in range(4):
                    qt = hp
                    ktf = 4 + hp
                    op0 = psmix.tile([128, 512], F32, name="op0", tag="mix")
                    op1 = psmix.tile([128, 512], F32, name="op1", tag="mix")
                    ops = (op0, op1)
                    for kt in range(NKT):
                        st = psST.tile([128, 1024], F32, name="st", tag="st")
                        for sub in range(2):
                            lo = sub * 64
                            nc.tensor.matmul(
                                st[:, sub * 512:(sub + 1) * 512],
                                lhsT=QTKT[lo:lo + 64, ktf * T + kt * 128: ktf * T + (kt + 1) * 128],
                                rhs=QTKT[lo:lo + 64, qt * T + qc * 512: qt * T + qc * 512 + 512],
                                start=True, stop=True)
                        pt = ptpool.tile([128, 1024], BF16, name="pt", tag="pt")
                        nc.scalar.activation(
                            pt[:], st[:], AF.Exp,
                            bias=BK[:, kt:kt + 1], scale=0.125)
                        for sub in range(2):
                            h = 2 * hp + sub
                            nc.tensor.matmul(
                                ops[sub][:],
                                lhsT=VHAT[:, kt * VROW + h * 128: kt * VROW + (h + 1) * 128],
                                rhs=pt[:, sub * 512:(sub + 1) * 512],
                                start=(kt == 0), stop=(kt == NKT - 1))
                    OTc = otpool.tile([128, 512], BF16, name="OTc", tag="otc")
                    for sub in range(2):
                        rec = recpool.tile([64, 512], F32, name="rec", tag="rec")
                        nc.vector.reciprocal_approx_fast(rec[:], ops[sub][0:64, :])
                        nc.vector.tensor_mul(
                            OTc[sub * 64:sub * 64 + 64, :],
                            ops[sub][64:128, :], rec[:])
                    otc.append(OTc)
                # output projection for this 512-token chunk
                for t4 in range(4):
                    tt = qc * 4 + t4
                    for c2 in range(2):
                        ps = psmix.tile([128, 512], F32, name="ps_y", tag="mix")
                        for f in range(4):
                            nc.tensor.matmul(
                                ps[:],
                                lhsT=otc[f][:, t4 * 128:(t4 + 1) * 128],
                                rhs=WO[:, f * 1024 + c2 * 512: f * 1024 + c2 * 512 + 512],
                                start=(f == 0), stop=(f == 3))
                        yv = evacpool.tile([128, 512], F32, name="yv", tag="yv")
                        nc.vector.tensor_add(yv[:], ps[:], BO[:, c2 * 512:(c2 + 1) * 512])
                        nc.sync.dma_start(
                            y_d[tt * 128:(tt + 1) * 128, c2 * 512:(c2 + 1) * 512], yv[:])

    nc.compile()
    return nc


def get_program():
    if "nc" not in _cache:
        _cache["nc"] = _build_program()
    return _cache["nc"]


def shard_inputs(x, engagement, mask, qkv_w, qkv_b, out_w, out_b):
    """Build the per-core input maps (host-side layout prep only)."""
    x = np.asarray(x, dtype=np.float32)
    engagement = np.asarray(engagement, dtype=np.float32)
    maskf = np.asarray(mask).astype(np.float32)
    qkv_w = np.asarray(qkv_w, dtype=np.float32)
    qkv_b = np.asarray(qkv_b, dtype=np.float32)
    out_w = np.asarray(out_w, dtype=np.float32)
    out_b = np.asarray(out_b, dtype=np.float32)

    qkvT = qkv_w.T  # [D, 3D]
    outT = out_w.T  # [D, D]
    in_maps = []
    for cix in range(8):
        b, hg = cix // 2, cix % 2
        qcols = qkvT[:, hg * 512:(hg + 1) * 512]
        kcols = qkvT[:, 1024 + hg * 512: 1024 + (hg + 1) * 512]
        sel = np.concatenate([qcols, kcols], axis=1)  # [1024 din, 1024 feats]
        # [d, p, m, f] -> [m, p, d, f] -> [(m p), (d f)]
        wqk = sel.reshape(NDT, 128, 8, 128).transpose(2, 1, 0, 3).reshape(1024, 1024)
        bq = qkv_b[hg * 512:(hg + 1) * 512].reshape(4, 128).T
        bk = qkv_b[1024 + hg * 512: 1024 + (hg + 1) * 512].reshape(4, 128).T
        bo = np.broadcast_to(out_b, (128, 1024)) if hg == 0 else np.zeros((128, 1024), np.float32)
        in_maps.append({
            "xt": np.ascontiguousarray(x[b].T).astype(NP_BF16),
            "wqk": np.ascontiguousarray(wqk).astype(NP_BF16),
            "wv": np.ascontiguousarray(
                qkvT[:, 2048 + hg * 512: 2048 + (hg + 1) * 512]).astype(NP_BF16),
            "bqk": np.ascontiguousarray(np.concatenate([bq, bk], axis=1)),
            "bv": np.ascontiguousarray(
                np.broadcast_to(qkv_b[2048 + hg * 512: 2048 + (hg + 1) * 512], (128, 512))),
            "eng": np.ascontiguousarray(engagement[b].reshape(NKT, 128).T),
            "maskf": np.ascontiguousarray(maskf[b].reshape(NKT, 128).T),
            "wo": np.ascontiguousarray(outT[hg * 512:(hg + 1) * 512, :]).astype(NP_BF16),
            "bo": np.ascontiguousarray(bo),
        })
    return in_maps


def kernel(x, engagement, mask, qkv_w, qkv_b, out_w, out_b):
    global last_results
    nc = get_program()
    in_maps = shard_inputs(x, engagement, mask, qkv_w, qkv_b, out_w, out_b)
    res = run_bass_kernel_spmd(nc, in_maps, list(range(8)))
    last_results = res
    out = np.empty((B, T, D), dtype=np.float32)
    for b in range(B):
        out[b] = res.results[2 * b]["y"] + res.results[2 * b + 1]["y"]
    return out


# revision 11
# speedup vs baseline: 1.1327x; 1.1225x over previous
"""EngagementBiasedMHA on 8 Trainium2 NeuronCores.

Sharding: 4 batches x 2 head-groups (8 heads each).  Each core computes, for
its (batch, head-group):
  - K^T projection in [feat, token] layout and V projection in [token, feat]
    layout (phase 1); V is stored per key-tile as [ones(64) | V_h] so the PV
    matmul also produces the softmax denominator on partitions 0:64
  - per 512-query chunk: Q^T projection (overlapped with attention of the
    previous chunk), then attention in transposed layout: S^T = K @ Q^T with
    keys on partitions, so the per-key engagement bias/mask folds into the
    Exp activation as a per-partition bias, and exp(S^T) is already the
    correct (lhs-contraction) layout for the PV matmul
  - O^T = Vhat^T @ P^T accumulated over key tiles (rows 0:64 = replicated
    softmax denominator, rows 64:128 = head output)
  - row-parallel partial output projection y_partial = O_hg @ out_w.T[hg]
Matmul operands are bf16 (4x PE throughput vs fp32); accumulation stays fp32.
Host side: transpose/slice inputs per core, then sum the two partial outputs
per batch (row-parallel unshard).
"""

import sys

if "/opt/trn_rl_repo" not in sys.path:
    sys.path.insert(0, "/opt/trn_rl_repo")

import numpy as np
from concourse import bacc, tile
import concourse.mybir as mybir
from concourse.bass_utils import run_bass_kernel_spmd

F32 = mybir.dt.float32
BF16 = mybir.dt.bfloat16
NP_BF16 = mybir.dt.np(BF16)
AF = mybir.ActivationFunctionType

B, T, D, H = 4, 2048, 1024, 16
HD = 64
HG = 8           # heads per core
NKT = T // 128   # 16 key/token tiles
NQC = T // 512   # 4 query chunks
NDT = D // 128   # 8 d_in tiles
VROW = HG * 128  # 1024 Vhat columns per key tile: per head [ones(64) | V(64)]

_cache = {}

# Results of the most recent run (for the test harness to read exec times).
last_results = None


def _build_program():
    nc = bacc.Bacc("TRN2", target_bir_lowering=False, debug=False, num_devices=8)
    xt_d = nc.declare_dram_parameter("xt", [D, T], BF16, isOutput=False)
    # wqk: row block m*128+p holds, at col d*128+f, weight qkv_w.T[d*128+p, feat(m)+f]
    wqk_d = nc.declare_dram_parameter("wqk", [1024, 1024], BF16, isOutput=False)
    wv_d = nc.declare_dram_parameter("wv", [D, 512], BF16, isOutput=False)
    bqk_d = nc.declare_dram_parameter("bqk", [128, 8], F32, isOutput=False)
    bv_d = nc.declare_dram_parameter("bv", [128, 512], F32, isOutput=False)
    eng_d = nc.declare_dram_parameter("eng", [128, NKT], F32, isOutput=False)
    maskf_d = nc.declare_dram_parameter("maskf", [128, NKT], F32, isOutput=False)
    wo_d = nc.declare_dram_parameter("wo", [512, 1024], BF16, isOutput=False)
    bo_d = nc.declare_dram_parameter("bo", [128, 1024], F32, isOutput=False)
    y_d = nc.declare_dram_parameter("y", [T, D], F32, isOutput=True)

    with tile.TileContext(nc) as tc:
        with (
            tc.tile_pool(name="persist", bufs=1) as persist,
            tc.tile_pool(name="wpool", bufs=1) as wpool,
            tc.tile_pool(name="small", bufs=1) as small,
            tc.tile_pool(name="ptpool", bufs=4) as ptpool,
            tc.tile_pool(name="otpool", bufs=9) as otpool,
            tc.tile_pool(name="evacpool", bufs=3) as evacpool,
            tc.tile_pool(name="recpool", bufs=3) as recpool,
            tc.tile_pool(name="evac2", bufs=2) as evac2,
            tc.tile_pool(name="psmix", bufs=4, space="PSUM") as psmix,
            tc.tile_pool(name="psST", bufs=2, space="PSUM") as psST,
        ):
            # ---- small inputs ----
            BQK = small.tile([128, 8], F32, name="BQK")
            nc.sync.dma_start(BQK[:], bqk_d[:])
            BV = small.tile([128, 512], F32, name="BV")
            nc.sync.dma_start(BV[:], bv_d[:])
            ENG = small.tile([128, NKT], F32, name="ENG")
            nc.sync.dma_start(ENG[:], eng_d[:])
            MSK = small.tile([128, NKT], F32, name="MSK")
            nc.sync.dma_start(MSK[:], maskf_d[:])
            BO = small.tile([128, 1024], F32, name="BO")
            nc.sync.dma_start(BO[:], bo_d[:])

            # ---- per-key bias: BK = ln(max(eng, 1e-6)) - 1e9 * mask ----
            BK = small.tile([128, NKT], F32, name="BK")
            nc.vector.tensor_scalar_max(BK[:], ENG[:], 1e-6)
            nc.scalar.activation(BK[:], BK[:], AF.Ln)
            MK9 = small.tile([128, NKT], F32, name="MK9")
            nc.vector.tensor_scalar_mul(MK9[:], MSK[:], -1e9)
            nc.vector.tensor_add(BK[:], BK[:], MK9[:])

            # ---- resident activations / weights (bf16) ----
            XT = persist.tile([128, NDT * T], BF16, name="XT")
            WQK = persist.tile([128, 8 * 1024], BF16, name="WQK")
            WV = wpool.tile([128, NDT * 512], BF16, name="WV", tag="wv_wo")
            # chunk 0 of x, K weights, and V weights first so compute starts early
            for d in range(NDT):
                nc.sync.dma_start(XT[:, d * T: d * T + 512],
                                  xt_d[d * 128:(d + 1) * 128, 0:512])
            for m in (4, 5, 6, 7, 0, 1, 2, 3):
                nc.sync.dma_start(WQK[:, m * 1024:(m + 1) * 1024],
                                  wqk_d[m * 128:(m + 1) * 128, :])
            for d in range(NDT):
                nc.sync.dma_start(WV[:, d * 512:(d + 1) * 512], wv_d[d * 128:(d + 1) * 128, :])
            for c in range(1, NQC):
                for d in range(NDT):
                    nc.sync.dma_start(XT[:, d * T + c * 512: d * T + (c + 1) * 512],
                                      xt_d[d * 128:(d + 1) * 128, c * 512:(c + 1) * 512])

            QTKT = persist.tile([128, 8 * T], BF16, name="QTKT")
            VHAT = persist.tile([128, NKT * VROW], BF16, name="VHAT")
            nc.gpsimd.memset(VHAT[:], 1.0)

            # ---- phase 1: K^T and V projections (chunked over tokens) ----
            for c in range(NQC):
                for m in range(4, 8):  # K feature tiles
                    ps = psmix.tile([128, 512], F32, name="ps_k", tag="mix")
                    for d in range(NDT):
                        nc.tensor.matmul(
                            ps[:],
                            lhsT=WQK[:, m * 1024 + d * 128: m * 1024 + (d + 1) * 128],
                            rhs=XT[:, d * T + c * 512: d * T + c * 512 + 512],
                            start=(d == 0), stop=(d == NDT - 1),
                        )
                    nc.scalar.activation(
                        QTKT[:, m * T + c * 512: m * T + c * 512 + 512],
                        ps[:], AF.Identity, bias=BQK[:, m:m + 1])
                for t4 in range(4):
                    t = c * 4 + t4
                    ps = psmix.tile([128, 512], F32, name="ps_v", tag="mix")
                    for d in range(NDT):
                        nc.tensor.matmul(
                            ps[:],
                            lhsT=XT[:, d * T + t * 128: d * T + (t + 1) * 128],
                            rhs=WV[:, d * 512:(d + 1) * 512],
                            start=(d == 0), stop=(d == NDT - 1),
                        )
                    vslice = VHAT[:, t * VROW:(t + 1) * VROW].rearrange(
                        "p (h c) -> p h c", c=128)[:, :, 64:128]
                    nc.vector.tensor_add(
                        vslice,
                        ps[:].rearrange("p (h c) -> p h c", c=64),
                        BV[:].rearrange("p (h c) -> p h c", c=64))

            WO = wpool.tile([128, 4 * 1024], BF16, name="WO", tag="wv_wo")
            for f in range(4):
                nc.sync.dma_start(WO[:, f * 1024:(f + 1) * 1024], wo_d[f * 128:(f + 1) * 128, :])

            # ---- phase 2: per query chunk: attention with Q-proj of the next
            # chunk and out-proj of the previous chunk interleaved into the
            # per-head-pair slack (PE is in-order; boundary matmuls must sit
            # where ACT has runway) ----
            def q_proj(qc2, m):
                ps = psmix.tile([128, 512], F32, name="ps_q", tag="mix")
                for d in range(NDT):
                    nc.tensor.matmul(
                        ps[:],
                        lhsT=WQK[:, m * 1024 + d * 128: m * 1024 + (d + 1) * 128],
                        rhs=XT[:, d * T + qc2 * 512: d * T + qc2 * 512 + 512],
                        start=(d == 0), stop=(d == NDT - 1),
                    )
                nc.vector.tensor_scalar_add(
                    QTKT[:, m * T + qc2 * 512: m * T + qc2 * 512 + 512],
                    ps[:], BQK[:, m:m + 1])

            def out_proj(qc2, otc2, grp):
                t4, c2 = grp // 2, grp % 2
                tt = qc2 * 4 + t4
                ps = psmix.tile([128, 512], F32, name="ps_y", tag="mix")
                for f in range(4):
                    nc.tensor.matmul(
                        ps[:],
                        lhsT=otc2[f][:, t4 * 128:(t4 + 1) * 128],
                        rhs=WO[:, f * 1024 + c2 * 512: f * 1024 + c2 * 512 + 512],
                        start=(f == 0), stop=(f == 3))
                yv = evacpool.tile([128, 512], F32, name="yv", tag="yv")
                nc.vector.tensor_add(yv[:], ps[:], BO[:, c2 * 512:(c2 + 1) * 512])
                nc.sync.dma_start(
                    y_d[tt * 128:(tt + 1) * 128, c2 * 512:(c2 + 1) * 512], yv[:])

            for m in range(4):
                q_proj(0, m)
            prev = None  # (qc, otc) awaiting out-projection
            for qc in range(NQC):
                otc = []
                for hp in range(4):
                    qt = hp
                    ktf = 4 + hp
                    op0 = psmix.tile([128, 512], F32, name="op0", tag="mix")
                    op1 = psmix.tile([128, 512], F32, name="op1", tag="mix")
                    ops = (op0, op1)
                    for kt in range(NKT):
                        st = psST.tile([128, 1024], F32, name="st", tag="st")
                        for sub in range(2):
                            lo = sub * 64
                            nc.tensor.matmul(
                                st[:, sub * 512:(sub + 1) * 512],
                                lhsT=QTKT[lo:lo + 64, ktf * T + kt * 128: ktf * T + (kt + 1) * 128],
                                rhs=QTKT[lo:lo + 64, qt * T + qc * 512: qt * T + qc * 512 + 512],
                                start=True, stop=True)
                        pt = ptpool.tile([128, 1024], BF16, name="pt", tag="pt")
                        nc.scalar.activation(
                            pt[:], st[:], AF.Exp,
                            bias=BK[:, kt:kt + 1], scale=0.125)
                        for sub in range(2):
                            h = 2 * hp + sub
                            nc.tensor.matmul(
                                ops[sub][:],
                                lhsT=VHAT[:, kt * VROW + h * 128: kt * VROW + (h + 1) * 128],
                                rhs=pt[:, sub * 512:(sub + 1) * 512],
                                start=(kt == 0), stop=(kt == NKT - 1))
                    # evacuate raw accumulators quickly to free the PSUM slots,
                    # normalize from SBUF afterwards
                    OTc = otpool.tile([128, 512], BF16, name="OTc", tag="otc")
                    for sub in range(2):
                        rec = recpool.tile([64, 512], F32, name="rec", tag="rec")
                        nc.vector.reciprocal_approx_fast(rec[:], ops[sub][0:64, :])
                        nc.vector.tensor_mul(
                            OTc[sub * 64:sub * 64 + 64, :],
                            ops[sub][64:128, :], rec[:])
                    otc.append(OTc)
                    # boundary work in the ACT runway after this head pair
                    if prev is not None:
                        out_proj(prev[0], prev[1], 2 * hp)
                        out_proj(prev[0], prev[1], 2 * hp + 1)
                    if qc + 1 < NQC:
                        q_proj(qc + 1, hp)
                prev = (qc, otc)
            for grp in range(8):
                out_proj(prev[0], prev[1], grp)
    nc.compile()
    return nc


def get_program():
    if "nc" not in _cache:
        _cache["nc"] = _build_program()
    return _cache["nc"]


def shard_inputs(x, engagement, mask, qkv_w, qkv_b, out_w, out_b):
    """Build the per-core input maps (host-side layout prep only)."""
    x = np.asarray(x, dtype=np.float32)
    engagement = np.asarray(engagement, dtype=np.float32)
    maskf = np.asarray(mask).astype(np.float32)
    qkv_w = np.asarray(qkv_w, dtype=np.float32)
    qkv_b = np.asarray(qkv_b, dtype=np.float32)
    out_w = np.asarray(out_w, dtype=np.float32)
    out_b = np.asarray(out_b, dtype=np.float32)

    qkvT = qkv_w.T  # [D, 3D]
    outT = out_w.T  # [D, D]
    in_maps = []
    for cix in range(8):
        b, hg = cix // 2, cix % 2
        qcols = qkvT[:, hg * 512:(hg + 1) * 512]
        kcols = qkvT[:, 1024 + hg * 512: 1024 + (hg + 1) * 512]
        sel = np.concatenate([qcols, kcols], axis=1)  # [1024 din, 1024 feats]
        # [d, p, m, f] -> [m, p, d, f] -> [(m p), (d f)]
        wqk = sel.reshape(NDT, 128, 8, 128).transpose(2, 1, 0, 3).reshape(1024, 1024)
        bq = qkv_b[hg * 512:(hg + 1) * 512].reshape(4, 128).T
        bk = qkv_b[1024 + hg * 512: 1024 + (hg + 1) * 512].reshape(4, 128).T
        bo = np.broadcast_to(out_b, (128, 1024)) if hg == 0 else np.zeros((128, 1024), np.float32)
        in_maps.append({
            "xt": np.ascontiguousarray(x[b].T).astype(NP_BF16),
            "wqk": np.ascontiguousarray(wqk).astype(NP_BF16),
            "wv": np.ascontiguousarray(
                qkvT[:, 2048 + hg * 512: 2048 + (hg + 1) * 512]).astype(NP_BF16),
            "bqk": np.ascontiguousarray(np.concatenate([bq, bk], axis=1)),
            "bv": np.ascontiguousarray(
                np.broadcast_to(qkv_b[2048 + hg * 512: 2048 + (hg + 1) * 512], (128, 512))),
            "eng": np.ascontiguousarray(engagement[b].reshape(NKT, 128).T),
            "maskf": np.ascontiguousarray(maskf[b].reshape(NKT, 128).T),
            "wo": np.ascontiguousarray(outT[hg * 512:(hg + 1) * 512, :]).astype(NP_BF16),
            "bo": np.ascontiguousarray(bo),
        })
    return in_maps


def kernel(x, engagement, mask, qkv_w, qkv_b, out_w, out_b):
    global last_results
    nc = get_program()
    in_maps = shard_inputs(x, engagement, mask, qkv_w, qkv_b, out_w, out_b)
    res = run_bass_kernel_spmd(nc, in_maps, list(range(8)))
    last_results = res
    out = np.empty((B, T, D), dtype=np.float32)
    for b in range(B):
        out[b] = res.results[2 * b]["y"] + res.results[2 * b + 1]["y"]
    return out
